# revision 1
# baseline (speedup 1.0000x reference)
"""Trainium2 Bass kernel for nn_ActorNetSpiking (spiking actor network).

Data-parallel over 8 NeuronCores: batch 4096 -> 512 per core.

Design:
- Feature-major layout on device: activations [feature_rows (partitions), batch] per
  timestep. Layer outputs row-ordered (l, co) i.e. row = l*Cout + co.
- All matmuls in bf16 (spike inputs are exact 0/1 in bf16); conv layers lowered to
  banded matmuls on host; PSUM accumulates fp32.
- Layer-decoupled time processing in blocks of Tb steps: layer l's whole block is
  computed before layer l+1 (spike history buffers carry the block).
- LIF per layer-step:
    PE   : syn_psum = W_chunks @ spike_hist[t]           (banded bf16 matmuls)
    ACT  : syn = Identity(psum + bias_pp)  (PSUM->SBUF drain, per-M-tile bias, bf16)
    DVE  : u  = (u * 0.5) + syn            (scalar_tensor_tensor)
    DVE  : v  = (vt * 0.75) + u            (scalar_tensor_tensor)
    DVE  : s  = (v > 0.5)                  (tensor_scalar -> bf16 spike history)
    GPS  : vt = (v <= 0.5) * v             (scalar_tensor_tensor, the LIF reset)
- fc4 layer (2 rows) accumulates spikes into acc; out = acc/50.
"""

import os
import numpy as np
import ml_dtypes

import concourse.bass as bass
import concourse.bacc as bacc_mod
import concourse.tile as tile
from concourse import mybir
from concourse._compat import with_exitstack
from concourse.bass_utils import run_bass_kernel_spmd

F32 = mybir.dt.float32
BF16 = mybir.dt.bfloat16
AF = mybir.ActivationFunctionType
OP = mybir.AluOpType

N_CORES = 8
B_FULL = 4096
B = B_FULL // N_CORES  # 512 per core
T = 50
Tb = 3  # time block

# layer geometry: (rows_in, rows_out)
# conv rows ordered (l, c): row = l*C + c
CONV = [  # (Lin, Lout, Cin, Cout)
    (360, 178, 1, 5),
    (178, 87, 5, 5),
    (87, 42, 5, 5),
]
FC = [(216, 256), (256, 256), (256, 128), (128, 2)]


def _build_banded(w, b, Lin, Lout, Cin, Cout):
    """Dense banded matrix [rows_in, rows_out] for stride-2 k=5 conv1d."""
    rows_in, rows_out = Lin * Cin, Lout * Cout
    Wd = np.zeros((rows_in, rows_out), np.float32)
    K = w.shape[2]
    for l in range(Lout):
        for k in range(K):
            li = 2 * l + k
            # Wd[li*Cin + ci, l*Cout + co] = w[co, ci, k]
            Wd[li * Cin:(li + 1) * Cin, l * Cout:(l + 1) * Cout] = w[:, :, k].T
    bias = np.tile(b, Lout)  # row (l, co) -> b[co]
    return Wd, bias


def _plan_layers(inp):
    """Host-side: per layer -> dict with M-tiles, K-chunks, lhsT data, biases."""
    mats = []
    for i, (Lin, Lout, Cin, Cout) in enumerate(CONV):
        w, b = inp[f'conv{i+1}_w'], inp[f'conv{i+1}_b']
        mats.append(_build_banded(w, b, Lin, Lout, Cin, Cout))
    # fc1 with column permutation: my row j (j<210) = (l3, co) -> ref idx co*42 + l3
    fw, fb = inp['fc1_w'], inp['fc1_b']
    Wd = np.zeros((216, 256), np.float32)
    for j in range(210):
        l3, co = j // 5, j % 5
        Wd[j, :] = fw[:, co * 42 + l3]
    Wd[210:216, :] = fw[:, 210:216].T
    mats.append((Wd, fb.copy()))
    for i in (2, 3, 4):
        fw, fb = inp[f'fc{i}_w'], inp[f'fc{i}_b']
        mats.append((fw.T.astype(np.float32), fb.copy()))

    layers = []
    for lidx, (Wd, bias) in enumerate(mats):
        rows_in, rows_out = Wd.shape
        # layers 1..6 consume ns (NOT-spike) history -> negate those columns
        # and absorb the rowsum into the per-step beta constant.
        # layer 0 input = raw scan spikes; fc1 rows 210..215 = raw normal spikes.
        ns_rows = np.zeros(rows_in, bool)
        if lidx >= 1:
            ns_rows[:] = True
            if lidx == 3:
                ns_rows[210:216] = False
        rowsum = (Wd.astype(np.float64) * ns_rows[:, None]).sum(axis=0)
        Wd = Wd.copy()
        Wd[ns_rows, :] *= -1.0
        tiles = []
        for m0 in range(0, rows_out, 128):
            m1 = min(m0 + 128, rows_out)
            nz = np.nonzero(np.any(Wd[:, m0:m1] != 0.0, axis=1))[0]
            k0, k1 = int(nz.min()), int(nz.max()) + 1
            chunks = []
            # chunks aligned to the 128-partition grid (matmul base partition
            # must be 0); leading/trailing zero rows in lhsT are free.
            for g in range(k0 // 128, (k1 + 127) // 128):
                a = g * 128
                bnd = min(a + 128, k1)
                chunks.append((a, bnd, Wd[a:bnd, m0:m1]))
            tiles.append(dict(m0=m0, m1=m1, chunks=chunks,
                              bias=bias[m0:m1], rowsum=rowsum[m0:m1]))
        layers.append(dict(rows_in=rows_in, rows_out=rows_out, tiles=tiles,
                           G_out=(rows_out + 127) // 128))
    return layers


def _pack_weights(layers):
    """Pack all lhsT chunks into one [128, total_cols] bf16 array + bias table."""
    cols = []
    total = 0
    for L in layers:
        for tl in L['tiles']:
            for (a, b_, Wc) in tl['chunks']:
                K, M = Wc.shape
                cols.append((total, Wc))
                total += M
    wpack = np.zeros((128, total), np.float32)
    off = 0
    offs = []
    for L in layers:
        for tl in L['tiles']:
            tl['offs'] = []
            for (a, b_, Wc) in tl['chunks']:
                K, M = Wc.shape
                wpack[:K, off:off + M] = Wc
                tl['offs'].append(off)
                off += M
    # beta tables: beta[r, t] = c_r * (2 - 2^(1-t)), t = 1..50 (step index+1)
    # where c_r = bias + rowsum over negated-input columns (ns-encoded inputs)
    ntiles = sum(len(L['tiles']) for L in layers)
    btab = np.zeros((128, ntiles * T), np.float32)
    ti = 0
    for L in layers:
        for tl in L['tiles']:
            tl['bidx'] = ti
            c = tl['bias'].astype(np.float64) + tl['rowsum']
            g = 2.0 - np.power(0.5, np.arange(T))  # t index 0..49 -> 2-2^(1-t), t=idx+1
            btab[:tl['m1'] - tl['m0'], ti * T:(ti + 1) * T] = (
                c[:, None] * g[None, :]).astype(np.float32)
            ti += 1
    hi = wpack.astype(ml_dtypes.bfloat16).astype(np.float32)
    mid = (wpack - hi).astype(ml_dtypes.bfloat16).astype(np.float32)
    lo = (wpack - hi - mid).astype(ml_dtypes.bfloat16)
    return np.concatenate([hi.astype(ml_dtypes.bfloat16),
                           mid.astype(ml_dtypes.bfloat16), lo], axis=1), btab


@with_exitstack
def _emit(ctx, tc, layers, wcols, nbt, prm):
    nc = tc.nc
    persist = ctx.enter_context(tc.tile_pool(name="persist", bufs=1))
    scanp = ctx.enter_context(tc.tile_pool(name="scanin", bufs=2))
    psum = ctx.enter_context(tc.tile_pool(name="psum", bufs=4, space="PSUM"))
    synp = ctx.enter_context(tc.tile_pool(name="syn", bufs=1))

    # weights + bias
    wsb = persist.tile([128, wcols], BF16, tag="wsb")
    nc.sync.dma_start(wsb[:], prm['w'][:])
    bsb = persist.tile([128, nbt], F32, tag="bsb")
    nc.sync.dma_start(bsb[:], prm['bias'][:])

    # merged spike-history buffer: 18 groups of [Tb, B] bf16 (layers 1..6)
    G_in = [3, 7, 4, 2, 2, 2, 1]
    hoff = [None, 0, 7, 11, 13, 15, 17]  # group offset per layer-input
    hist_all = persist.tile([128, 18, Tb, B], BF16, tag="hist")
    hist = [None] + [hist_all[:, hoff[li]:hoff[li] + G_in[li]]
                     for li in range(1, len(layers))]

    # merged states: u, vt each [128, 19*B] fp32, per-layer B-offset views
    goffs = []
    tot = 0
    for L in layers:
        goffs.append(tot)
        tot += L['G_out']
    u_all = persist.tile([128, tot * B], F32, tag="u")
    vt_all = persist.tile([128, tot * B], F32, tag="vt")
    us = [u_all[:, goffs[i] * B:(goffs[i] + L['G_out']) * B]
          for i, L in enumerate(layers)]
    vts = [vt_all[:, goffs[i] * B:(goffs[i] + L['G_out']) * B]
           for i, L in enumerate(layers)]
    acc = persist.tile([2, B], F32, tag="acc")
    ns4 = persist.tile([2, B], BF16, tag="ns4")

    nblocks = (T + Tb - 1) // Tb
    for blk in range(nblocks):
        t0 = blk * Tb
        tb = min(Tb, T - t0)
        # DMA conv1 input block: scan [360, T, B] -> hist[0]
        sc = scanp.tile([128, 3, Tb, B], BF16, tag="scan")
        for g in range(3):
            p = min(128, 360 - g * 128)
            nc.sync.dma_start(sc[:p, g, :tb, :],
                              prm['scan'][g * 128:g * 128 + p, t0:t0 + tb, :])
        # DMA normal spikes into fc1 input rows 210..215 (group 1, parts 82..88)
        nc.sync.dma_start(hist[3][82:88, 1, :tb, :],
                          prm['normal'][:, t0:t0 + tb, :])

        for li, L in enumerate(layers):
            h_in = sc if li == 0 else hist[li]
            g_out = L['G_out']
            tiles = L['tiles']
            u, vt = us[li], vts[li]
            v = synp.tile([128, g_out * B], F32, tag="v")
            for t in range(tb):
                t_abs = t0 + t
                for ph in range(0, len(tiles), 2):
                    grp = tiles[ph:ph + 2]
                    gw = len(grp)
                    ps = psum.tile([128, gw * B], F32, tag="ps")
                    for si, tl in enumerate(grp):
                        M = tl['m1'] - tl['m0']
                        nch = len(tl['chunks'])
                        for ci_, ((a, b_, Wc), off) in enumerate(
                                zip(tl['chunks'], tl['offs'])):
                            K = b_ - a
                            g_src, p_src = a // 128, a % 128
                            for half in range(3):
                                nc.tensor.matmul(
                                    ps[:M, si * B:(si + 1) * B],
                                    wsb[:K, half * (wcols // 3) + off:
                                        half * (wcols // 3) + off + Wc.shape[1]],
                                    h_in[p_src:p_src + K, g_src, t, :],
                                    start=(ci_ == 0 and half == 0),
                                    stop=(ci_ == nch - 1 and half == 2))
                    # u = 0.5*u + syn   (reads PSUM directly)
                    if t_abs == 0:
                        nc.vector.tensor_scalar(
                            u[:, ph * B:(ph + gw) * B], ps[:], 1.0, None,
                            op0=OP.mult)
                    else:
                        nc.vector.scalar_tensor_tensor(
                            u[:, ph * B:(ph + gw) * B],
                            u[:, ph * B:(ph + gw) * B],
                            0.5, ps[:], op0=OP.mult, op1=OP.add)
                # v = 0.75*vt + u
                if t_abs == 0:
                    nc.vector.tensor_scalar(v[:], u, 1.0, None, op0=OP.mult)
                else:
                    nc.vector.scalar_tensor_tensor(
                        v[:], vt, 0.75, u, op0=OP.mult, op1=OP.add)
                # per-tile threshold (ns) + reset, with per-(row,t) beta
                for ti, tl in enumerate(tiles):
                    M = tl['m1'] - tl['m0']
                    col = tl['bidx'] * T + t_abs
                    beta = bsb[:M, col:col + 1]
                    if li < 6:
                        nsout = hist[li + 1][:M, ti, t, :]
                    else:
                        nsout = ns4[:M, :]
                    nc.vector.tensor_scalar(
                        nsout, v[:M, ti * B:(ti + 1) * B], beta, 0.5,
                        op0=OP.add, op1=OP.is_le)
                    nc.vector.scalar_tensor_tensor(
                        vt[:M, ti * B:(ti + 1) * B],
                        v[:M, ti * B:(ti + 1) * B], beta, nsout,
                        op0=OP.add, op1=OP.mult)
                if li == 6:
                    # acc_neg += ns4 - 1   (acc_neg = -sum(spikes))
                    if t_abs == 0:
                        nc.vector.tensor_scalar(
                            acc[:], ns4[:2, :], 1.0, None, op0=OP.subtract)
                    else:
                        nc.vector.scalar_tensor_tensor(
                            acc[:], ns4[:2, :], 1.0, acc[:],
                            op0=OP.subtract, op1=OP.add)

    out_sb = persist.tile([2, B], F32, tag="outsb")
    nc.vector.tensor_scalar_mul(out_sb[:], acc[:], -1.0 / T)
    nc.sync.dma_start(prm['out'][:], out_sb[:])


_CACHE = {}


def _get_nc(layers, wcols, nbt):
    key = ('nc', wcols, nbt)
    if key in _CACHE:
        return _CACHE[key]
    nc = bacc_mod.Bacc()
    prm = dict(
        scan=nc.declare_dram_parameter("scan", [360, T * B], BF16,
                                       isOutput=False).rearrange(
                                           "l (t b) -> l t b", b=B),
        normal=nc.declare_dram_parameter("normal", [6, T * B], BF16,
                                         isOutput=False).rearrange(
                                             "l (t b) -> l t b", b=B),
        w=nc.declare_dram_parameter("w", [128, wcols], BF16, isOutput=False),
        bias=nc.declare_dram_parameter("bias", [128, nbt], F32, isOutput=False),
        out=nc.declare_dram_parameter("out", [2, B], F32, isOutput=True),
    )
    with tile.TileContext(nc) as tc:
        _emit(tc, layers, wcols, nbt, prm)
    nc.compile()
    _CACHE[key] = nc
    return nc


def kernel(normal_spikes, scan_spikes, batch_size,
           conv1_w, conv1_b, conv2_w, conv2_b, conv3_w, conv3_b,
           fc1_w, fc1_b, fc2_w, fc2_b, fc3_w, fc3_b, fc4_w, fc4_b):
    inp = dict(conv1_w=conv1_w, conv1_b=conv1_b, conv2_w=conv2_w,
               conv2_b=conv2_b, conv3_w=conv3_w, conv3_b=conv3_b,
               fc1_w=fc1_w, fc1_b=fc1_b, fc2_w=fc2_w, fc2_b=fc2_b,
               fc3_w=fc3_w, fc3_b=fc3_b, fc4_w=fc4_w, fc4_b=fc4_b)
    inp = {k: np.asarray(v, np.float32) for k, v in inp.items()}
    layers = _plan_layers(inp)
    wpack, btab = _pack_weights(layers)
    wcols, nbt = wpack.shape[1], btab.shape[1]

    nc = _get_nc(layers, wcols, nbt)

    bf = ml_dtypes.bfloat16
    # scan_spikes (4096, 1, 360, 50) -> per core [360, 50, 512] bf16
    scan_t = np.ascontiguousarray(
        np.asarray(scan_spikes)[:, 0].transpose(1, 2, 0)).astype(bf)  # [360,50,4096]
    norm_t = np.ascontiguousarray(
        np.asarray(normal_spikes).transpose(1, 2, 0)).astype(bf)  # [6,50,4096]

    in_maps = []
    for c in range(N_CORES):
        sl = slice(c * B, (c + 1) * B)
        in_maps.append(dict(
            scan=np.ascontiguousarray(scan_t[:, :, sl]).reshape(360, T * B),
            normal=np.ascontiguousarray(norm_t[:, :, sl]).reshape(6, T * B),
            w=wpack, bias=btab))
    import time as _time
    t0 = _time.time()
    res = run_bass_kernel_spmd(nc, in_maps, list(range(N_CORES)))
    wall1 = _time.time() - t0
    outs = [res.results[c]["out"] for c in range(N_CORES)]
    full = np.concatenate([o.T for o in outs], axis=0).astype(np.float32)
    kernel._last_exec_ns = res.exec_time_ns
    kernel._wall_exec_s = wall1
    return full

